# revision 27
# baseline (speedup 1.0000x reference)
"""PerformerAttention (softmax attention + interleaved RoPE) on 8 trn2 cores.

Sharding: data-parallel over batch (16 batches -> 2 per core), weights replicated.

v6 design (all-bf16, transposed attention, software-pipelined):
  qk^T = wqk^T.T @ x^T computed for BOTH local batches in one pass
    (x tiles hold 2*577 token columns; RoPE runs over the merged width with
    cos=1/sin=0 at the two CLS columns so no special-casing).
  S^T = k^T q per head ([keys, queries]); P^T = exp(S^T) (no max subtraction,
    logits bounded ~|8.7|); out^T/sums = [V | 1]^T @ P^T with a ones column
    interleaved in V so softmax row-sums appear as psum row 64 for free.
  Softmax denominators: sums rows DMA'd onto partitions 0..11 of one tile;
    ONE Ln + ONE Exp(scale=-1) per batch (2 ACT table loads), then per-head
    stream_shuffle(mask=[h]*32) broadcasts 1/sums for the normalize multiply.
  Software pipeline: within each batch's head-pair loop, the READY out^T
    matmuls of pair hp-1 are issued before the (EXP-WAR-gated) S^T matmuls of
    pair hp so the PE FIFO never idles on the scalar engine; V(batch1) and
    proj(batch0) are interleaved nt-chunk-wise into the two attention loops.
"""

import numpy as np
import ml_dtypes

import concourse.bass as bass
import concourse.mybir as mybir
import concourse.tile as tile
from concourse import bacc
from concourse.bass_utils import run_bass_kernel_spmd

F32 = mybir.dt.float32
BF16 = mybir.dt.bfloat16
COPY = mybir.ActivationFunctionType.Copy
EXP = mybir.ActivationFunctionType.Exp
LN = mybir.ActivationFunctionType.Ln

B, N, C, H, D = 16, 577, 768, 12, 64
NCORES = 8
BPC = B // NCORES  # batches per core
N2 = BPC * N  # 1154 merged token columns
NT = [(0, 128), (128, 128), (256, 128), (384, 128), (512, 65)]  # token tiles
NCH = [(0, 512), (512, 65)]   # per-batch token chunks (psum bank = 512 fp32)
QCH = [(0, 512), (512, 512), (1024, 130)]  # merged-qk token chunks
FCH = [(0, 512), (512, 256)]  # feature free-dim chunks
VW = 65 * H  # 780: V stored as 12 blocks of [v_h | ones]

_CACHED_NC = None
LAST_RESULTS = None  # test harness reads exec_time_ns off this


def _build_nc():
    nc = bacc.Bacc("TRN2", target_bir_lowering=False)

    xT_d = nc.dram_tensor("xT", [C, N2], BF16, kind="ExternalInput")
    wqk_d = nc.dram_tensor("wqkT", [C, 2 * C], BF16, kind="ExternalInput")
    wv_d = nc.dram_tensor("wvT", [C, C], BF16, kind="ExternalInput")
    wp_d = nc.dram_tensor("wpT", [C, C], BF16, kind="ExternalInput")
    c_d = nc.dram_tensor("c2", [128, N2], BF16, kind="ExternalInput")
    s_d = nc.dram_tensor("s2", [128, N2], BF16, kind="ExternalInput")
    bias_d = nc.dram_tensor("biasb", [128, C], F32, kind="ExternalInput")
    out_d = nc.dram_tensor("out", [BPC, N, C], F32, kind="ExternalOutput")

    with tile.TileContext(nc) as tc:
        with (
            tc.tile_pool(name="const", bufs=1) as constp,
            tc.tile_pool(name="xp", bufs=1) as xp,
            tc.tile_pool(name="qkp", bufs=1) as qkp,
            tc.tile_pool(name="vp", bufs=1) as vp,
            tc.tile_pool(name="atp", bufs=2) as atp,
            tc.tile_pool(name="work", bufs=2) as work,
            tc.tile_pool(name="pep", bufs=2) as pep,
            tc.tile_pool(name="rp", bufs=2) as rp,
            tc.tile_pool(name="ocp", bufs=1) as ocp,
            tc.tile_pool(name="obp", bufs=2) as obp,
            tc.tile_pool(name="psA", bufs=2, space="PSUM") as psA,
            tc.tile_pool(name="psB", bufs=2, space="PSUM") as psB,
        ):
            # ---- constants / weights (once per core) ----
            wqk = []
            wv = []
            wp = []
            for ct in range(6):
                t = constp.tile([128, 2 * C], BF16, name=f"wqk{ct}", tag=f"wqk{ct}")
                nc.sync.dma_start(t, wqk_d[ct * 128:(ct + 1) * 128, :])
                wqk.append(t)
                t = constp.tile([128, C], BF16, name=f"wv{ct}", tag=f"wv{ct}")
                nc.sync.dma_start(t, wv_d[ct * 128:(ct + 1) * 128, :])
                wv.append(t)
                t = constp.tile([128, C], BF16, name=f"wp{ct}", tag=f"wp{ct}")
                nc.sync.dma_start(t, wp_d[ct * 128:(ct + 1) * 128, :])
                wp.append(t)
            c2 = constp.tile([128, N2], BF16, name="c2", tag="c2")
            nc.sync.dma_start(c2, c_d[:, :])
            s2 = constp.tile([128, N2], BF16, name="s2", tag="s2")
            nc.sync.dma_start(s2, s_d[:, :])
            biasb = constp.tile([128, C], F32, name="biasb", tag="biasb")
            nc.sync.dma_start(biasb, bias_d[:, :])

            # ---- load merged x^T tiles (both batches side by side) ----
            xT = []
            for ct in range(6):
                t = xp.tile([128, N2], BF16, name=f"xT{ct}", tag=f"xT{ct}")
                nc.sync.dma_start(t, xT_d[ct * 128:(ct + 1) * 128, :])
                xT.append(t)

            # ---- qk^T for both batches + RoPE -> bf16 qkT [128, 1154] ----
            qkT = []
            for ft in range(12):
                psq = []
                for qi in range(2):
                    psq.append(psA.tile([128, 1024], F32, name="ps_qk",
                                        tag="psA"))
                for ci, (cs, cw) in enumerate(QCH):
                    dst = psq[0][:, cs:cs + cw] if ci < 2 else psq[1][:, 0:cw]
                    for ct in range(6):
                        nc.tensor.matmul(
                            dst,
                            lhsT=wqk[ct][:, ft * 128:(ft + 1) * 128],
                            rhs=xT[ct][:, cs:cs + cw],
                            start=(ct == 0), stop=(ct == 5),
                        )
                wkt = work.tile([128, N2], BF16, name="wkt", tag="wkt")
                nc.scalar.activation(wkt[:, 0:1024], psq[0][:, 0:1024], COPY,
                                     bias=0.0, scale=1.0)
                nc.scalar.activation(wkt[:, 1024:N2], psq[1][:, 0:130], COPY,
                                     bias=0.0, scale=1.0)
                qk = qkp.tile([128, N2], BF16, name=f"qkT{ft}", tag=f"qkT{ft}")
                tsw = work.tile([128, N2], BF16, name="tsw", tag="tsw")
                # tsw = [to0; te0; to1; te1] (swap 32-row even/odd blocks)
                nc.vector.tensor_copy(tsw[0:32, :], wkt[32:64, :])
                nc.vector.tensor_copy(tsw[32:64, :], wkt[0:32, :])
                nc.vector.tensor_copy(tsw[64:96, :], wkt[96:128, :])
                nc.vector.tensor_copy(tsw[96:128, :], wkt[64:96, :])
                nc.vector.tensor_mul(qk, wkt, c2)  # CLS cols: c=1
                nc.vector.tensor_mul(tsw, tsw, s2)  # s2 = [-s;s;-s;s], CLS: 0
                nc.vector.tensor_add(qk, qk, tsw)
                qkT.append(qk)

            # ---- V builder (per batch, per token tile) ----
            V = {0: [None] * 5, 1: [None] * 5}

            def build_v(b, nt):
                ns, nsz = NT[nt]
                vt = vp.tile([128, VW], BF16, name=f"V{b}_{nt}",
                             tag=f"V{b}_{nt}")
                ones_ap = vt.rearrange("p (h d) -> p h d", d=65)[:, :, 64:65]
                nc.vector.memset(ones_ap, 1.0)
                ps = psB.tile([128, 1024], F32, name="ps_v", tag="psB")
                for (fs, fw) in FCH:
                    for ct in range(6):
                        nc.tensor.matmul(
                            ps[0:nsz, fs:fs + fw],
                            lhsT=xT[ct][:, b * N + ns:b * N + ns + nsz],
                            rhs=wv[ct][:, fs:fs + fw],
                            start=(ct == 0), stop=(ct == 5),
                        )
                dst = vt[0:nsz, :].rearrange("p (h d) -> p h d", d=65)[:, :, 0:64]
                src = ps[0:nsz, 0:C].rearrange("p (h d) -> p h d", d=64)
                nc.vector.tensor_copy(dst, src)
                V[b][nt] = vt

            # ---- proj builder (per batch, per token tile) ----
            ATTs = {}

            def build_proj(b, nt):
                ns, nsz = NT[nt]
                ATT = ATTs[b]
                ps = psB.tile([128, 1024], F32, name="ps_p", tag="psB")
                for (fs, fw) in FCH:
                    for ct in range(6):
                        nc.tensor.matmul(
                            ps[0:nsz, fs:fs + fw],
                            lhsT=ATT[ct][:, ns:ns + nsz],
                            rhs=wp[ct][:, fs:fs + fw],
                            start=(ct == 0), stop=(ct == 5),
                        )
                ob = obp.tile([128, C], F32, name="ob", tag="ob")
                nc.vector.tensor_add(ob[0:nsz, :], ps[0:nsz, 0:C],
                                     biasb[0:nsz, :])
                nc.sync.dma_start(out_d[b, ns:ns + nsz, :], ob[0:nsz, :])

            # ---- attention for one batch, side work interleaved ----
            def attention(b, side_work):
                ssum = rp.tile([128, N], BF16, name="ssum", tag="ssum")
                nc.vector.memset(ssum[0:32, :], 1.0)
                ATT = []
                OUTC = []
                prev_pe = None
                prev_psO = None
                for hp in range(7):
                    if hp >= 1:
                        psO = prev_psO
                    if hp < 6:
                        at = atp.tile([128, N], BF16, name=f"attnT{hp}",
                                      tag=f"attnT{hp}")
                        ATT.append(at)
                        Pe = {0: [], 1: []}
                    for mi, (ms, msz) in enumerate(NT):
                        # out^T for pair hp-1 first: its inputs are ready, so
                        # the PE FIFO always has work while S^T waits on EXP
                        if hp >= 1:
                            for hi in range(2):
                                h = 2 * (hp - 1) + hi
                                for (cs, cw) in NCH:
                                    nc.tensor.matmul(
                                        psO[hi][0:65, cs:cs + cw],
                                        lhsT=V[b][mi][0:msz,
                                                      h * 65:h * 65 + 65],
                                        rhs=prev_pe[hi][mi][0:msz, cs:cs + cw],
                                        start=(mi == 0), stop=(mi == 4),
                                    )
                        if hp < 6:
                            psS = {}
                            for hi in range(2):
                                psS[hi] = psA.tile([128, 1024], F32,
                                                   name="ps_s", tag="psA")
                            for (cs, cw) in NCH:
                                for hi in range(2):
                                    h = 2 * hp + hi
                                    qt = qkT[h // 2][hi * 64:hi * 64 + 64, :]
                                    kt = qkT[6 + h // 2][hi * 64:hi * 64 + 64, :]
                                    nc.tensor.matmul(
                                        psS[hi][0:msz, cs:cs + cw],
                                        lhsT=kt[:, b * N + ms:b * N + ms + msz],
                                        rhs=qt[:, b * N + cs:b * N + cs + cw],
                                        start=True, stop=True,
                                    )
                            for hi in range(2):
                                pe = pep.tile([128, N], BF16, name="pe",
                                              tag=f"pe{hi}_{ms}")
                                nc.scalar.activation(pe[0:msz, 0:N],
                                                     psS[hi][0:msz, 0:N], EXP,
                                                     bias=0.0, scale=1.0)
                                Pe[hi].append(pe)
                    if hp >= 1:
                        # drain pair hp-1: evacuate + stage sums row
                        for hi in range(2):
                            h = 2 * (hp - 1) + hi
                            outc = ocp.tile([128, N], BF16, name=f"outc{h}",
                                            tag=f"outc{h}")
                            nc.vector.tensor_copy(outc[0:65, 0:N],
                                                  psO[hi][0:65, 0:N])
                            OUTC.append(outc)
                            nc.sync.dma_start(ssum[h:h + 1, 0:N],
                                              outc[64:65, 0:N])
                        if side_work and (hp - 1) < len(side_work):
                            side_work[hp - 1]()
                    if hp < 6:
                        prev_psO = [psB.tile([128, 1024], F32, name="ps_o",
                                             tag="psB") for _ in range(2)]
                        prev_pe = Pe

                # batched softmax denominators: 1/s = exp(-ln(s))
                nc.scalar.activation(ssum[0:32, :], ssum[0:32, :], LN,
                                     bias=0.0, scale=1.0)
                nc.scalar.activation(ssum[0:32, :], ssum[0:32, :], EXP,
                                     bias=0.0, scale=-1.0)
                for h in range(12):
                    recb = rp.tile([64, N], BF16, name="recb", tag="recb")
                    bmask = [h] * 32
                    nc.vector.stream_shuffle(recb[0:32, :], ssum[0:32, :],
                                             bmask)
                    nc.vector.stream_shuffle(recb[32:64, :], ssum[0:32, :],
                                             bmask)
                    nc.vector.tensor_mul(
                        ATT[h // 2][(h % 2) * 64:(h % 2) * 64 + 64, 0:N],
                        OUTC[h][0:64, 0:N], recb[0:64, :])
                ATTs[b] = ATT

            # ---- schedule ----
            for nt in range(5):
                build_v(0, nt)
            attention(0, [lambda nt=nt: build_v(1, nt) for nt in range(5)])
            attention(1, [lambda nt=nt: build_proj(0, nt) for nt in range(5)])
            for nt in range(5):
                build_proj(1, nt)

    nc.compile()
    return nc


def _rope_perm():
    idx = []
    for h in range(H):
        base = h * D
        idx.extend(base + 2 * i for i in range(D // 2))      # evens
        idx.extend(base + 2 * i + 1 for i in range(D // 2))  # odds
    return np.array(idx)


def _prep_inputs(x, wqkv, wproj, bproj, freqs_cos, freqs_sin):
    perm = _rope_perm()
    wq = wqkv[0:C][perm] * 0.125
    wk = wqkv[C:2 * C][perm]
    wqkT = np.ascontiguousarray(
        np.concatenate([wq, wk], axis=0).T).astype(ml_dtypes.bfloat16)
    wvT = np.ascontiguousarray(wqkv[2 * C:].T).astype(ml_dtypes.bfloat16)
    wpT = np.ascontiguousarray(wproj.T).astype(ml_dtypes.bfloat16)
    cosT = np.ascontiguousarray(freqs_cos.T, dtype=np.float32)  # [32, 576]
    sinT = np.ascontiguousarray(freqs_sin.T, dtype=np.float32)
    c128 = np.concatenate([cosT] * 4, axis=0)          # [128, 576]
    s128 = np.concatenate([-sinT, sinT, -sinT, sinT], axis=0)
    c2 = np.ones((128, N2), dtype=np.float32)
    s2 = np.zeros((128, N2), dtype=np.float32)
    for b in range(BPC):
        c2[:, b * N + 1:(b + 1) * N] = c128
        s2[:, b * N + 1:(b + 1) * N] = s128
    c2 = c2.astype(ml_dtypes.bfloat16)
    s2 = s2.astype(ml_dtypes.bfloat16)
    biasb = np.broadcast_to(bproj.astype(np.float32), (128, C)).copy()

    in_maps = []
    for core in range(NCORES):
        xs = x[core * BPC:(core + 1) * BPC]
        xT = np.concatenate([np.asarray(xs[b]).T for b in range(BPC)],
                            axis=1)
        in_maps.append({
            "xT": np.ascontiguousarray(xT).astype(ml_dtypes.bfloat16),
            "wqkT": wqkT,
            "wvT": wvT,
            "wpT": wpT,
            "c2": c2,
            "s2": s2,
            "biasb": biasb,
        })
    return in_maps


def kernel(x, wqkv, wproj, bproj, freqs_cos, freqs_sin, trace=False):
    global _CACHED_NC, LAST_RESULTS
    if _CACHED_NC is None:
        _CACHED_NC = _build_nc()
    in_maps = _prep_inputs(x, wqkv, wproj, bproj, freqs_cos, freqs_sin)
    res = run_bass_kernel_spmd(_CACHED_NC, in_maps,
                               core_ids=list(range(NCORES)), trace=trace)
    LAST_RESULTS = res
    out = np.concatenate([r["out"] for r in res.results], axis=0)
    return out.astype(np.float32)


# revision 33
# speedup vs baseline: 1.0737x; 1.0737x over previous
"""PerformerAttention (softmax attention + interleaved RoPE) on 8 trn2 cores.

Sharding: data-parallel over batch (16 batches -> 2 per core), weights replicated.

v6 design (all-bf16, transposed attention, software-pipelined):
  qk^T = wqk^T.T @ x^T computed for BOTH local batches in one pass
    (x tiles hold 2*577 token columns; RoPE runs over the merged width with
    cos=1/sin=0 at the two CLS columns so no special-casing).
  S^T = k^T q per head ([keys, queries]); P^T = exp(S^T) (no max subtraction,
    logits bounded ~|8.7|); out^T/sums = [V | 1]^T @ P^T with a ones column
    interleaved in V so softmax row-sums appear as psum row 64 for free.
  Softmax denominators: sums rows DMA'd onto partitions 0..11 of one tile;
    ONE Ln + ONE Exp(scale=-1) per batch (2 ACT table loads), then per-head
    stream_shuffle(mask=[h]*32) broadcasts 1/sums for the normalize multiply.
  Software pipeline: within each batch's head-pair loop, the READY out^T
    matmuls of pair hp-1 are issued before the (EXP-WAR-gated) S^T matmuls of
    pair hp so the PE FIFO never idles on the scalar engine; V(batch1) and
    proj(batch0) are interleaved nt-chunk-wise into the two attention loops.
"""

import numpy as np
import ml_dtypes

import concourse.bass as bass
import concourse.mybir as mybir
import concourse.tile as tile
from concourse import bacc
from concourse.bass_utils import run_bass_kernel_spmd

F32 = mybir.dt.float32
BF16 = mybir.dt.bfloat16
COPY = mybir.ActivationFunctionType.Copy
EXP = mybir.ActivationFunctionType.Exp
LN = mybir.ActivationFunctionType.Ln

B, N, C, H, D = 16, 577, 768, 12, 64
NCORES = 8
BPC = B // NCORES  # batches per core
N2 = BPC * N  # 1154 merged token columns
NT = [(0, 128), (128, 128), (256, 128), (384, 128), (512, 65)]  # token tiles
NCH = [(0, 512), (512, 65)]   # per-batch token chunks (psum bank = 512 fp32)
QCH = [(0, 512), (512, 512), (1024, 130)]  # merged-qk token chunks
FCH = [(0, 512), (512, 256)]  # feature free-dim chunks
VW = 65 * H  # 780: V stored as 12 blocks of [v_h | ones]

_CACHED_NC = None
LAST_RESULTS = None  # test harness reads exec_time_ns off this


def _build_nc():
    nc = bacc.Bacc("TRN2", target_bir_lowering=False)

    xT_d = nc.dram_tensor("xT", [C, N2], BF16, kind="ExternalInput")
    wqk_d = nc.dram_tensor("wqkT", [C, 2 * C], BF16, kind="ExternalInput")
    wv_d = nc.dram_tensor("wvT", [C, C], BF16, kind="ExternalInput")
    wp_d = nc.dram_tensor("wpT", [C, C], BF16, kind="ExternalInput")
    c_d = nc.dram_tensor("c2", [128, N2], BF16, kind="ExternalInput")
    s_d = nc.dram_tensor("s2", [128, N2], BF16, kind="ExternalInput")
    bias_d = nc.dram_tensor("biasb", [128, C], F32, kind="ExternalInput")
    out_d = nc.dram_tensor("out", [BPC, N, C], F32, kind="ExternalOutput")

    with tile.TileContext(nc) as tc:
        with (
            tc.tile_pool(name="const", bufs=1) as constp,
            tc.tile_pool(name="xp", bufs=1) as xp,
            tc.tile_pool(name="qkp", bufs=1) as qkp,
            tc.tile_pool(name="vp", bufs=1) as vp,
            tc.tile_pool(name="atp", bufs=2) as atp,
            tc.tile_pool(name="work", bufs=2) as work,
            tc.tile_pool(name="pep", bufs=2) as pep,
            tc.tile_pool(name="rp", bufs=2) as rp,
            tc.tile_pool(name="ocp", bufs=1) as ocp,
            tc.tile_pool(name="obp", bufs=2) as obp,
            tc.tile_pool(name="psA", bufs=2, space="PSUM") as psA,
            tc.tile_pool(name="psB", bufs=4, space="PSUM") as psB,
        ):
            # ---- constants / weights (once per core) ----
            wqk = []
            wv = []
            wp = []
            for ct in range(6):
                t = constp.tile([128, 2 * C], BF16, name=f"wqk{ct}", tag=f"wqk{ct}")
                nc.sync.dma_start(t, wqk_d[ct * 128:(ct + 1) * 128, :])
                wqk.append(t)
                t = constp.tile([128, C], BF16, name=f"wv{ct}", tag=f"wv{ct}")
                nc.sync.dma_start(t, wv_d[ct * 128:(ct + 1) * 128, :])
                wv.append(t)
                t = constp.tile([128, C], BF16, name=f"wp{ct}", tag=f"wp{ct}")
                nc.sync.dma_start(t, wp_d[ct * 128:(ct + 1) * 128, :])
                wp.append(t)
            c2 = constp.tile([128, N2], BF16, name="c2", tag="c2")
            nc.sync.dma_start(c2, c_d[:, :])
            s2 = constp.tile([128, N2], BF16, name="s2", tag="s2")
            nc.sync.dma_start(s2, s_d[:, :])
            biasb = constp.tile([128, C], F32, name="biasb", tag="biasb")
            nc.sync.dma_start(biasb, bias_d[:, :])

            # ---- load merged x^T tiles (both batches side by side) ----
            xT = []
            for ct in range(6):
                t = xp.tile([128, N2], BF16, name=f"xT{ct}", tag=f"xT{ct}")
                nc.sync.dma_start(t, xT_d[ct * 128:(ct + 1) * 128, :])
                xT.append(t)

            # ---- qk^T for both batches + RoPE -> bf16 qkT [128, 1154] ----
            qkT = []
            for ft in range(12):
                psq = []
                for qi in range(2):
                    psq.append(psA.tile([128, 1024], F32, name="ps_qk",
                                        tag="psA"))
                for ci, (cs, cw) in enumerate(QCH):
                    dst = psq[0][:, cs:cs + cw] if ci < 2 else psq[1][:, 0:cw]
                    for ct in range(6):
                        nc.tensor.matmul(
                            dst,
                            lhsT=wqk[ct][:, ft * 128:(ft + 1) * 128],
                            rhs=xT[ct][:, cs:cs + cw],
                            start=(ct == 0), stop=(ct == 5),
                        )
                wkt = work.tile([128, N2], BF16, name="wkt", tag="wkt")
                nc.scalar.activation(wkt[:, 0:1024], psq[0][:, 0:1024], COPY,
                                     bias=0.0, scale=1.0)
                nc.scalar.activation(wkt[:, 1024:N2], psq[1][:, 0:130], COPY,
                                     bias=0.0, scale=1.0)
                qk = qkp.tile([128, N2], BF16, name=f"qkT{ft}", tag=f"qkT{ft}")
                tsw = work.tile([128, N2], BF16, name="tsw", tag="tsw")
                # tsw = [to0; te0; to1; te1] (swap 32-row even/odd blocks)
                nc.vector.tensor_copy(tsw[0:32, :], wkt[32:64, :])
                nc.vector.tensor_copy(tsw[32:64, :], wkt[0:32, :])
                nc.vector.tensor_copy(tsw[64:96, :], wkt[96:128, :])
                nc.vector.tensor_copy(tsw[96:128, :], wkt[64:96, :])
                nc.vector.tensor_mul(qk, wkt, c2)  # CLS cols: c=1
                nc.vector.tensor_mul(tsw, tsw, s2)  # s2 = [-s;s;-s;s], CLS: 0
                nc.vector.tensor_add(qk, qk, tsw)
                qkT.append(qk)

            # ---- V builder (per batch, per token tile) ----
            V = {0: [None] * 5, 1: [None] * 5}

            def build_v(b, nt):
                ns, nsz = NT[nt]
                vt = vp.tile([128, VW], BF16, name=f"V{b}_{nt}",
                             tag=f"V{b}_{nt}")
                ones_ap = vt.rearrange("p (h d) -> p h d", d=65)[:, :, 64:65]
                nc.vector.memset(ones_ap, 1.0)
                nh = {0: 8, 512: 4}  # heads per feature chunk
                for (fs, fw) in FCH:
                    ps = psB.tile([128, 512], F32, name="ps_v", tag="psB")
                    for ct in range(6):
                        nc.tensor.matmul(
                            ps[0:nsz, 0:fw],
                            lhsT=xT[ct][:, b * N + ns:b * N + ns + nsz],
                            rhs=wv[ct][:, fs:fs + fw],
                            start=(ct == 0), stop=(ct == 5),
                        )
                    h0 = fs // 64
                    dst = vt[0:nsz, h0 * 65:(h0 + nh[fs]) * 65].rearrange(
                        "p (h d) -> p h d", d=65)[:, :, 0:64]
                    src = ps[0:nsz, 0:fw].rearrange("p (h d) -> p h d", d=64)
                    nc.vector.tensor_copy(dst, src)
                V[b][nt] = vt

            # ---- proj builder (per batch, per token tile) ----
            ATTs = {}

            def build_proj(b, nt):
                ns, nsz = NT[nt]
                ATT = ATTs[b]
                ob = obp.tile([128, C], F32, name="ob", tag="ob")
                for (fs, fw) in FCH:
                    ps = psB.tile([128, 512], F32, name="ps_p", tag="psB")
                    for ct in range(6):
                        nc.tensor.matmul(
                            ps[0:nsz, 0:fw],
                            lhsT=ATT[ct][:, ns:ns + nsz],
                            rhs=wp[ct][:, fs:fs + fw],
                            start=(ct == 0), stop=(ct == 5),
                        )
                    nc.vector.tensor_add(ob[0:nsz, fs:fs + fw],
                                         ps[0:nsz, 0:fw],
                                         biasb[0:nsz, fs:fs + fw])
                nc.sync.dma_start(out_d[b, ns:ns + nsz, :], ob[0:nsz, :])

            # ---- attention for one batch, side work interleaved ----
            def attention(b, side_work):
                ssum = rp.tile([128, N], BF16, name="ssum", tag="ssum")
                nc.vector.memset(ssum[0:32, :], 1.0)
                ATT = []
                OUTC = []
                prev_pe = None
                prev_psO = None
                for hp in range(7):
                    if hp >= 1:
                        psO = prev_psO
                    if hp < 6:
                        at = atp.tile([128, N], BF16, name=f"attnT{hp}",
                                      tag=f"attnT{hp}")
                        ATT.append(at)
                        Pe = {0: [], 1: []}
                    for mi, (ms, msz) in enumerate(NT):
                        # out^T for pair hp-1 first: its inputs are ready, so
                        # the PE FIFO always has work while S^T waits on EXP
                        if hp >= 1:
                            for hi in range(2):
                                h = 2 * (hp - 1) + hi
                                for ci, (cs, cw) in enumerate(NCH):
                                    nc.tensor.matmul(
                                        psO[hi][ci][0:65, 0:cw],
                                        lhsT=V[b][mi][0:msz,
                                                      h * 65:h * 65 + 65],
                                        rhs=prev_pe[hi][mi][0:msz, cs:cs + cw],
                                        start=(mi == 0), stop=(mi == 4),
                                    )
                        if hp < 6:
                            psS = {}
                            for hi in range(2):
                                psS[hi] = psA.tile([128, 1024], F32,
                                                   name="ps_s", tag="psA")
                            for (cs, cw) in NCH:
                                for hi in range(2):
                                    h = 2 * hp + hi
                                    qt = qkT[h // 2][hi * 64:hi * 64 + 64, :]
                                    kt = qkT[6 + h // 2][hi * 64:hi * 64 + 64, :]
                                    nc.tensor.matmul(
                                        psS[hi][0:msz, cs:cs + cw],
                                        lhsT=kt[:, b * N + ms:b * N + ms + msz],
                                        rhs=qt[:, b * N + cs:b * N + cs + cw],
                                        start=True, stop=True,
                                    )
                            for hi in range(2):
                                pe = pep.tile([128, N], BF16, name="pe",
                                              tag=f"pe{hi}_{ms}")
                                nc.scalar.activation(pe[0:msz, 0:N],
                                                     psS[hi][0:msz, 0:N], EXP,
                                                     bias=0.0, scale=1.0)
                                Pe[hi].append(pe)
                    if hp >= 1:
                        # drain pair hp-1: evacuate + stage sums row
                        for hi in range(2):
                            h = 2 * (hp - 1) + hi
                            outc = ocp.tile([128, N], BF16, name=f"outc{h}",
                                            tag=f"outc{h}")
                            nc.vector.tensor_copy(outc[0:65, 0:512],
                                                  psO[hi][0][0:65, 0:512])
                            nc.vector.tensor_copy(outc[0:65, 512:N],
                                                  psO[hi][1][0:65, 0:65])
                            OUTC.append(outc)
                            nc.sync.dma_start(ssum[h:h + 1, 0:N],
                                              outc[64:65, 0:N])
                        if side_work and (hp - 1) < len(side_work):
                            side_work[hp - 1]()
                    if hp < 6:
                        prev_psO = [
                            [psB.tile([128, 512], F32, name="ps_o", tag="psB")
                             for _ in NCH] for _ in range(2)]
                        prev_pe = Pe

                # batched softmax denominators: 1/s = exp(-ln(s))
                nc.scalar.activation(ssum[0:32, :], ssum[0:32, :], LN,
                                     bias=0.0, scale=1.0)
                nc.scalar.activation(ssum[0:32, :], ssum[0:32, :], EXP,
                                     bias=0.0, scale=-1.0)
                for h in range(12):
                    recb = rp.tile([64, N], BF16, name="recb", tag="recb")
                    bmask = [h] * 32
                    nc.vector.stream_shuffle(recb[0:32, :], ssum[0:32, :],
                                             bmask)
                    nc.vector.stream_shuffle(recb[32:64, :], ssum[0:32, :],
                                             bmask)
                    nc.vector.tensor_mul(
                        ATT[h // 2][(h % 2) * 64:(h % 2) * 64 + 64, 0:N],
                        OUTC[h][0:64, 0:N], recb[0:64, :])
                ATTs[b] = ATT

            # ---- schedule ----
            for nt in range(5):
                build_v(0, nt)
            attention(0, [lambda nt=nt: build_v(1, nt) for nt in range(5)])
            attention(1, [lambda nt=nt: build_proj(0, nt) for nt in range(5)])
            for nt in range(5):
                build_proj(1, nt)

    nc.compile()
    return nc


def _rope_perm():
    idx = []
    for h in range(H):
        base = h * D
        idx.extend(base + 2 * i for i in range(D // 2))      # evens
        idx.extend(base + 2 * i + 1 for i in range(D // 2))  # odds
    return np.array(idx)


def _prep_inputs(x, wqkv, wproj, bproj, freqs_cos, freqs_sin):
    perm = _rope_perm()
    wq = wqkv[0:C][perm] * 0.125
    wk = wqkv[C:2 * C][perm]
    wqkT = np.ascontiguousarray(
        np.concatenate([wq, wk], axis=0).T).astype(ml_dtypes.bfloat16)
    wvT = np.ascontiguousarray(wqkv[2 * C:].T).astype(ml_dtypes.bfloat16)
    wpT = np.ascontiguousarray(wproj.T).astype(ml_dtypes.bfloat16)
    cosT = np.ascontiguousarray(freqs_cos.T, dtype=np.float32)  # [32, 576]
    sinT = np.ascontiguousarray(freqs_sin.T, dtype=np.float32)
    c128 = np.concatenate([cosT] * 4, axis=0)          # [128, 576]
    s128 = np.concatenate([-sinT, sinT, -sinT, sinT], axis=0)
    c2 = np.ones((128, N2), dtype=np.float32)
    s2 = np.zeros((128, N2), dtype=np.float32)
    for b in range(BPC):
        c2[:, b * N + 1:(b + 1) * N] = c128
        s2[:, b * N + 1:(b + 1) * N] = s128
    c2 = c2.astype(ml_dtypes.bfloat16)
    s2 = s2.astype(ml_dtypes.bfloat16)
    biasb = np.broadcast_to(bproj.astype(np.float32), (128, C)).copy()

    in_maps = []
    for core in range(NCORES):
        xs = x[core * BPC:(core + 1) * BPC]
        xT = np.concatenate([np.asarray(xs[b]).T for b in range(BPC)],
                            axis=1)
        in_maps.append({
            "xT": np.ascontiguousarray(xT).astype(ml_dtypes.bfloat16),
            "wqkT": wqkT,
            "wvT": wvT,
            "wpT": wpT,
            "c2": c2,
            "s2": s2,
            "biasb": biasb,
        })
    return in_maps


def kernel(x, wqkv, wproj, bproj, freqs_cos, freqs_sin, trace=False):
    global _CACHED_NC, LAST_RESULTS
    if _CACHED_NC is None:
        _CACHED_NC = _build_nc()
    in_maps = _prep_inputs(x, wqkv, wproj, bproj, freqs_cos, freqs_sin)
    res = run_bass_kernel_spmd(_CACHED_NC, in_maps,
                               core_ids=list(range(NCORES)), trace=trace)
    LAST_RESULTS = res
    out = np.concatenate([r["out"] for r in res.results], axis=0)
    return out.astype(np.float32)
